# revision 1
# baseline (speedup 1.0000x reference)
"""RNN-T Joiner kernel for 8 Trainium2 NeuronCores.

out[b,t,u,:] = tanh(enc[b,t,:] + pred[b,u,:]) @ W.T + b

Sharding: data-parallel over t (400 -> 50 per core). Each core:
  - DVE: broadcast-add encT[:,t] + predT[:,u] -> logitT [c, (t,u)]
  - ACT: tanh in place (rounding to f32r for the PE)
  - PE:  psum[cells, v] += logitT[c, cells].T @ WT[c, v]  (float32r, N=512)
  - DVE: out_sbuf = psum + bias (replicated), DMA 256KB contiguous to DRAM

All constants (W.T, bias, enc slice, pred) are packed host-side into one
[128, 4960] tensor loaded by a single DMA so downstream instructions only
wait on one DMA-queue semaphore (walrus rejects >N sync waits per instr).
"""

import sys

sys.path.insert(0, "/opt/trn_rl_repo")

import numpy as np

import concourse.bass as bass
import concourse.bacc as bacc
import concourse.mybir as mybir
from concourse.tile import TileContext
from concourse.bass_utils import run_bass_kernel_spmd

B, T, U, C, V = 4, 400, 100, 512, 512
NCORES = 8
TS = T // NCORES  # 50 t per core
P = 128
CK = C // P  # 4 chunks of the contraction dim
CELLS = TS * U  # 5000 cells (t,u) per batch per core
BLK = 2  # logit blocks per batch
BCELLS = CELLS // BLK  # 2500
BT = TS // BLK  # 25 t per block
F32 = mybir.dt.float32
F32R = mybir.dt.float32r

# packed consts layout (columns of the [128, NCOL] tensor)
W_OFF = 0  # [ck, v] -> 4*512
BIAS_OFF = W_OFF + CK * V  # [v] replicated across partitions
ENC_OFF = BIAS_OFF + V  # [ck, b, t] -> 4*4*50
PRED_OFF = ENC_OFF + CK * B * TS  # [ck, b, u] -> 4*4*100
NCOL = PRED_OFF + CK * B * U  # 4960

_cache = {}


def _build():
    # Bacc (not raw Bass): its compile() runs generate_event_semaphores,
    # which splits >1-wait sync conditions that walrus rejects.
    nc = bacc.Bacc("TRN2", target_bir_lowering=False, debug=False)
    consts = nc.declare_dram_parameter("consts", [P, NCOL], F32, isOutput=False)
    out = nc.declare_dram_parameter("out", [B, TS, U, V], F32, isOutput=True)

    with TileContext(nc) as tc:
        with (
            tc.tile_pool(name="consts", bufs=1) as cpool,
            tc.tile_pool(name="logit", bufs=3) as logit_pool,
            tc.tile_pool(name="osb", bufs=8) as out_pool,
            tc.tile_pool(name="psum", bufs=8, space="PSUM") as psum_pool,
        ):
            cs = cpool.tile([P, NCOL], F32, tag="cs")
            nc.sync.dma_start(out=cs, in_=consts.ap())

            wview = cs[:, W_OFF : W_OFF + CK * V].rearrange(
                "p (ck v) -> p ck v", ck=CK
            )
            bias_sb = cs[:, BIAS_OFF : BIAS_OFF + V]
            eview = cs[:, ENC_OFF : ENC_OFF + CK * B * TS].rearrange(
                "p (ck b t) -> p ck b t", ck=CK, b=B
            )
            pview = cs[:, PRED_OFF : PRED_OFF + CK * B * U].rearrange(
                "p (ck b u) -> p ck b u", ck=CK, b=B
            )

            wt = []
            for ck in range(CK):
                # walrus requires f32r matmul operands to be rounded to f32r
                # by their producing instruction, so bounce through a DVE copy
                w_ = cpool.tile([P, V], F32R, tag=f"wt{ck}")
                nc.vector.tensor_copy(out=w_[:], in_=wview[:, ck, :])
                wt.append(w_)

            # cell tiles within a block: full 128-rows plus a 68-row tail
            tiles = [(s, P) for s in range(0, (BCELLS // P) * P, P)]
            rem = BCELLS - (BCELLS // P) * P
            if rem:
                tiles.append(((BCELLS // P) * P, rem))

            for b in range(B):
                ob = out.ap()[b].rearrange("t u v -> (t u) v")  # [5000, 512]
                for blk in range(BLK):
                    t0 = blk * BT
                    c0 = blk * BCELLS
                    lg = []
                    for ck in range(CK):
                        lgt = logit_pool.tile([P, BT, U], F32, tag=f"lg{ck}")
                        # Producer load-balance, measured: ACT-fused per-t ops
                        # cost ~9.9us/unit all-ACT; DVE-add+big-tanh costs
                        # ~2.6us DVE + ~2.4us ACT. With DVE also doing the
                        # output bias-adds (~110us), putting ~1/3 of units on
                        # the ACT path equalizes both engines (~165us).
                        unit = (b * BLK + blk) * CK + ck
                        if unit % 3 == 0:
                            # fused add+tanh on ACT, one op per t (bias is
                            # per-partition, fixed per op); f32r-rounded out
                            for t in range(BT):
                                nc.scalar.activation(
                                    out=lgt[:, t, :].bitcast(F32R),
                                    in_=pview[:, ck, b, :],
                                    func=mybir.ActivationFunctionType.Tanh,
                                    bias=eview[:, ck, b, t0 + t : t0 + t + 1],
                                )
                        else:
                            e_col = (
                                eview[:, ck, b, t0 : t0 + BT]
                                .unsqueeze(2)
                                .broadcast_to([P, BT, U])
                            )
                            p_row = (
                                pview[:, ck, b, :]
                                .unsqueeze(1)
                                .broadcast_to([P, BT, U])
                            )
                            nc.vector.tensor_add(
                                out=lgt[:].bitcast(F32R), in0=e_col, in1=p_row
                            )
                            nc.scalar.activation(
                                out=lgt[:].bitcast(F32R),
                                in_=lgt[:],
                                func=mybir.ActivationFunctionType.Tanh,
                            )
                        lg.append(lgt)
                    for s, m in tiles:
                        ps = psum_pool.tile([P, V], F32, tag="ps")
                        for ck in range(CK):
                            lgflat = lg[ck][:].rearrange("p t u -> p (t u)")
                            nc.tensor.matmul(
                                ps[:m, :],
                                lhsT=lgflat[:, s : s + m].bitcast(F32R),
                                rhs=wt[ck][:],
                                start=(ck == 0),
                                stop=(ck == CK - 1),
                            )
                        osb = out_pool.tile([P, V], F32, tag="osb")
                        nc.vector.tensor_add(
                            out=osb[:m], in0=ps[:m, :], in1=bias_sb[:m]
                        )
                        nc.sync.dma_start(out=ob[c0 + s : c0 + s + m, :], in_=osb[:m])
    nc.compile()
    return nc


def _install_ntff_hook():
    """This image's antenv lacks axon_hooks, so bass_utils' trace=True path
    can't find the NTFF profile hook. Inject the module and wire the ctypes
    hook from trn_boot against the axon PJRT .so."""
    if "antenv.axon_hooks" in sys.modules:
        return
    import types

    holder = [None]
    mod = types.ModuleType("antenv.axon_hooks")
    mod.set_axon_ntff_profile_hook = lambda h: holder.__setitem__(0, h)
    mod.get_axon_ntff_profile_hook = lambda: holder[0]
    sys.modules["antenv.axon_hooks"] = mod
    try:
        sys.path.insert(0, "/root/.axon_site/trn_agent_boot")
        from trn_boot import _ntff_profile_via_ctypes

        mod.set_axon_ntff_profile_hook(
            _ntff_profile_via_ctypes("/opt/axon/libaxon_pjrt.so")
        )
    except Exception as e:  # degrade to no tracing
        print(f"NTFF hook install failed: {e}", file=sys.stderr)


def _run(in_maps, trace=False, tmpdir=None):
    if "nc" not in _cache:
        _cache["nc"] = _build()
    if trace:
        _install_ntff_hook()
    return run_bass_kernel_spmd(
        _cache["nc"], in_maps, list(range(NCORES)), trace=trace, tmpdir=tmpdir
    )


def make_in_maps(encoder_out, predictor_out, W, b):
    encoder_out = np.asarray(encoder_out, dtype=np.float32)
    predictor_out = np.asarray(predictor_out, dtype=np.float32)
    W = np.asarray(W, dtype=np.float32)
    b = np.asarray(b, dtype=np.float32)

    base = np.empty((P, NCOL), np.float32)
    # [p, ck, v] <- W[v, ck*P+p]
    base[:, W_OFF : W_OFF + CK * V] = (
        W.reshape(V, CK, P).transpose(2, 1, 0).reshape(P, CK * V)
    )
    base[:, BIAS_OFF : BIAS_OFF + V] = np.broadcast_to(b, (P, V))
    # [p, ck, b, u] <- pred[b, u, ck*P+p]
    base[:, PRED_OFF : PRED_OFF + CK * B * U] = (
        predictor_out.reshape(B, U, CK, P).transpose(3, 2, 0, 1).reshape(P, -1)
    )

    in_maps = []
    for i in range(NCORES):
        m = base.copy()
        enc_s = encoder_out[:, i * TS : (i + 1) * TS, :]  # [b, t, c]
        m[:, ENC_OFF : ENC_OFF + CK * B * TS] = (
            enc_s.reshape(B, TS, CK, P).transpose(3, 2, 0, 1).reshape(P, -1)
        )
        in_maps.append({"consts": m})
    return in_maps


def kernel(encoder_out, predictor_out, W, b):
    in_maps = make_in_maps(encoder_out, predictor_out, W, b)
    res = _run(in_maps, trace=False)
    return np.concatenate(
        [res.results[i]["out"] for i in range(NCORES)], axis=1
    )



# revision 2
# speedup vs baseline: 1.0842x; 1.0842x over previous
"""RNN-T Joiner kernel for 8 Trainium2 NeuronCores.

out[b,t,u,:] = tanh(enc[b,t,:] + pred[b,u,:]) @ W.T + b

Sharding: data-parallel over t (400 -> 50 per core). Each core:
  - GPSIMD/DVE: broadcast-add encT[:,t] + predT[:,u] -> logitT [c, (t,u)] bf16
  - ACT: tanh in place (bf16)
  - PE:  psum[cells, v] += logitT[c, cells].T @ WT[c, v]  (bf16 operands, N=512)
  - DVE: out_sbuf(bf16) = psum(f32) + bias, DMA 128KB contiguous to DRAM
  - host: upcast bf16 -> f32 during the gather

Engine budget per core (measured rates): PE 640 MM x ~220ns = ~141us is the
critical path; DVE (psum copies + a few adds) ~130us, GPSIMD (most adds,
~2cyc/col two-input floor) ~120us, ACT (all tanh, big ops) ~77us, DMA out
bf16 20.5MB ~50us. bf16 operands keep rel err ~5e-3 << 2e-2 gate.

All constants (W.T, bias, enc slice, pred) are packed host-side into one
bf16 [128, 4960] tensor loaded by a single DMA so downstream instructions
only wait on one DMA-queue semaphore (walrus rejects >N sync waits per
instr).
"""

import sys

sys.path.insert(0, "/opt/trn_rl_repo")

import ml_dtypes
import numpy as np

import concourse.bass as bass
import concourse.bacc as bacc
import concourse.mybir as mybir
from concourse.tile import TileContext
from concourse.bass_utils import run_bass_kernel_spmd

B, T, U, C, V = 4, 400, 100, 512, 512
NCORES = 8
TS = T // NCORES  # 50 t per core
P = 128
CK = C // P  # 4 chunks of the contraction dim
CELLS = TS * U  # 5000 cells (t,u) per batch per core
BLK = 2  # logit blocks per batch
BCELLS = CELLS // BLK  # 2500
BT = TS // BLK  # 25 t per block
F32 = mybir.dt.float32
BF16 = mybir.dt.bfloat16

# packed consts layout (columns of the bf16 [128, NCOL] tensor)
W_OFF = 0  # [ck, v] -> 4*512
BIAS_OFF = W_OFF + CK * V  # [v] replicated across partitions
ENC_OFF = BIAS_OFF + V  # [ck, b, t] -> 4*4*50
PRED_OFF = ENC_OFF + CK * B * TS  # [ck, b, u] -> 4*4*100
NCOL = PRED_OFF + CK * B * U  # 4960

# producer add engine split: unit in 0..31; DVE takes these, GPSIMD the rest
DVE_ADD_UNITS = frozenset(u for u in range(B * BLK * CK) if u % 4 == 0)

_cache = {}


def _build():
    # Bacc (not raw Bass): its compile() runs generate_event_semaphores,
    # which splits >1-wait sync conditions that walrus rejects.
    nc = bacc.Bacc("TRN2", target_bir_lowering=False, debug=False)
    consts = nc.declare_dram_parameter("consts", [P, NCOL], BF16, isOutput=False)
    out = nc.declare_dram_parameter("out", [B, TS, U, V], BF16, isOutput=True)

    with TileContext(nc) as tc:
        with (
            tc.tile_pool(name="consts", bufs=1) as cpool,
            tc.tile_pool(name="logit", bufs=3) as logit_pool,
            tc.tile_pool(name="osb", bufs=8) as out_pool,
            tc.tile_pool(name="psum", bufs=8, space="PSUM") as psum_pool,
        ):
            cs = cpool.tile([P, NCOL], BF16, tag="cs")
            nc.sync.dma_start(out=cs, in_=consts.ap())

            wview = cs[:, W_OFF : W_OFF + CK * V].rearrange(
                "p (ck v) -> p ck v", ck=CK
            )
            bias_bf = cs[:, BIAS_OFF : BIAS_OFF + V]
            eview = cs[:, ENC_OFF : ENC_OFF + CK * B * TS].rearrange(
                "p (ck b t) -> p ck b t", ck=CK, b=B
            )
            pview = cs[:, PRED_OFF : PRED_OFF + CK * B * U].rearrange(
                "p (ck b u) -> p ck b u", ck=CK, b=B
            )

            # psum(f32)+bias tensor_tensor wants matching input dtypes
            bias_f32 = cpool.tile([P, V], F32, tag="bias_f32")
            nc.vector.tensor_copy(out=bias_f32[:], in_=bias_bf)

            # cell tiles within a block: full 128-rows plus a 68-row tail
            tiles = [(s, P) for s in range(0, (BCELLS // P) * P, P)]
            rem = BCELLS - (BCELLS // P) * P
            if rem:
                tiles.append(((BCELLS // P) * P, rem))

            for b in range(B):
                ob = out.ap()[b].rearrange("t u v -> (t u) v")  # [5000, 512]
                for blk in range(BLK):
                    t0 = blk * BT
                    c0 = blk * BCELLS
                    lg = []
                    for ck in range(CK):
                        lgt = logit_pool.tile([P, BT, U], BF16, tag=f"lg{ck}")
                        e_col = (
                            eview[:, ck, b, t0 : t0 + BT]
                            .unsqueeze(2)
                            .broadcast_to([P, BT, U])
                        )
                        p_row = (
                            pview[:, ck, b, :]
                            .unsqueeze(1)
                            .broadcast_to([P, BT, U])
                        )
                        unit = (b * BLK + blk) * CK + ck
                        eng = nc.vector if unit in DVE_ADD_UNITS else nc.gpsimd
                        eng.tensor_add(out=lgt[:], in0=e_col, in1=p_row)
                        nc.scalar.activation(
                            out=lgt[:],
                            in_=lgt[:],
                            func=mybir.ActivationFunctionType.Tanh,
                        )
                        lg.append(lgt)
                    for s, m in tiles:
                        ps = psum_pool.tile([P, V], F32, tag="ps")
                        for ck in range(CK):
                            lgflat = lg[ck][:].rearrange("p t u -> p (t u)")
                            nc.tensor.matmul(
                                ps[:m, :],
                                lhsT=lgflat[:, s : s + m],
                                rhs=wview[:, ck, :],
                                start=(ck == 0),
                                stop=(ck == CK - 1),
                            )
                        osb = out_pool.tile([P, V], BF16, tag="osb")
                        nc.vector.tensor_add(
                            out=osb[:m], in0=ps[:m, :], in1=bias_f32[:m]
                        )
                        nc.sync.dma_start(out=ob[c0 + s : c0 + s + m, :], in_=osb[:m])
    nc.compile()
    return nc


def _install_ntff_hook():
    """This image's antenv lacks axon_hooks, so bass_utils' trace=True path
    can't find the NTFF profile hook. Inject the module and wire the ctypes
    hook from trn_boot against the axon PJRT .so."""
    if "antenv.axon_hooks" in sys.modules:
        return
    import types

    holder = [None]
    mod = types.ModuleType("antenv.axon_hooks")
    mod.set_axon_ntff_profile_hook = lambda h: holder.__setitem__(0, h)
    mod.get_axon_ntff_profile_hook = lambda: holder[0]
    sys.modules["antenv.axon_hooks"] = mod
    try:
        sys.path.insert(0, "/root/.axon_site/trn_agent_boot")
        from trn_boot import _ntff_profile_via_ctypes

        mod.set_axon_ntff_profile_hook(
            _ntff_profile_via_ctypes("/opt/axon/libaxon_pjrt.so")
        )
    except Exception as e:  # degrade to no tracing
        print(f"NTFF hook install failed: {e}", file=sys.stderr)


def _run(in_maps, trace=False, tmpdir=None):
    if "nc" not in _cache:
        _cache["nc"] = _build()
    if trace:
        _install_ntff_hook()
    return run_bass_kernel_spmd(
        _cache["nc"], in_maps, list(range(NCORES)), trace=trace, tmpdir=tmpdir
    )


def make_in_maps(encoder_out, predictor_out, W, b):
    encoder_out = np.asarray(encoder_out, dtype=np.float32)
    predictor_out = np.asarray(predictor_out, dtype=np.float32)
    W = np.asarray(W, dtype=np.float32)
    b = np.asarray(b, dtype=np.float32)

    base = np.empty((P, NCOL), ml_dtypes.bfloat16)
    # [p, ck, v] <- W[v, ck*P+p]
    base[:, W_OFF : W_OFF + CK * V] = (
        W.reshape(V, CK, P).transpose(2, 1, 0).reshape(P, CK * V)
    )
    base[:, BIAS_OFF : BIAS_OFF + V] = np.broadcast_to(b, (P, V))
    # [p, ck, b, u] <- pred[b, u, ck*P+p]
    base[:, PRED_OFF : PRED_OFF + CK * B * U] = (
        predictor_out.reshape(B, U, CK, P).transpose(3, 2, 0, 1).reshape(P, -1)
    )

    in_maps = []
    for i in range(NCORES):
        m = base.copy()
        enc_s = encoder_out[:, i * TS : (i + 1) * TS, :]  # [b, t, c]
        m[:, ENC_OFF : ENC_OFF + CK * B * TS] = (
            enc_s.reshape(B, TS, CK, P).transpose(3, 2, 0, 1).reshape(P, -1)
        )
        in_maps.append({"consts": m})
    return in_maps


def kernel(encoder_out, predictor_out, W, b):
    in_maps = make_in_maps(encoder_out, predictor_out, W, b)
    res = _run(in_maps, trace=False)
    return np.concatenate(
        [np.asarray(res.results[i]["out"], dtype=np.float32) for i in range(NCORES)],
        axis=1,
    )
